# revision 66
# baseline (speedup 1.0000x reference)
"""Trainium2 Bass kernel for nn_ConditionalJiTBlock (DiT-style block with
AdaLN modulation, self-attention, cross-attention and SwiGLU FFN).

Sharding: 8 NeuronCores = 4 batch elements x 2 token-halves. Each core
computes its 512 query tokens end-to-end with zero collectives; the K/V
projections (which need all 1024 tokens of the batch element) are
replicated within each pair of cores. SPMD safety: the host permutes each
core's token axis so the core's local tokens are always columns 0..511 of
the on-chip tensors (attention is permutation-invariant over key tokens).

Layout: activations are feature-major on chip (features on partitions,
tokens on the free axis). Projections run as fp8 DoubleRow matmuls
(weights host-packed into [kp, 128, 2, F] pairs scaled by 2^10,
activations written as fp8 k-pair tiles; the 1/2^10 is folded into the
PSUM evacuation). Attention scores for a head pair are emitted as two
row-tiled concurrent K=64 matmuls into one 2-bank [128, 1024] PSUM tile,
exponentiated by a single wide ACT op. Softmax denominators come from an
interleaved ones-column in the token-major V tiles (PSUM row 64 of each
head's P@V output). The AdaLN mods matvec is chunked by modulation group
(shift/scale groups 0-2 gate stage 1; groups 3-8 stream during stage 1)
and transposed to feature-major on chip with PE transposes.
"""

import numpy as np
import ml_dtypes

BF16 = ml_dtypes.bfloat16
F8 = ml_dtypes.float8_e4m3

B, N, M, D, H, HD = 4, 1024, 1024, 1024, 16, 64
MH = 2730
MHP = 2816          # MH padded to 22*128
EPS = 1e-6
NCORES = 8
T = 512             # local query tokens per core
DT = D // 128       # 8
KP = DT // 2        # 4 k-tile pairs for DoubleRow
FHT = MHP // 128    # 22
NMOD = 9
ATT_SCALE = HD ** -0.5
WS = 1024.0         # fp8 weight pre-scale (power of 2)
IWS = 1.0 / WS

# fp8-DoubleRow enables, per projection
FP8 = dict(qkv=True, wo=True, cq=True, ckv=True, co=True, ffn=True)
DBG = False


# ==========================================================================
# device graph
# ==========================================================================

def build_graph():
    import concourse.bacc as bacc
    import concourse.mybir as mybir
    import concourse.tile as tile

    F32 = mybir.dt.float32
    BT = mybir.dt.bfloat16
    E4 = mybir.dt.float8e4

    nc = bacc.Bacc("TRN2", target_bir_lowering=False, debug=False,
                   num_devices=NCORES)

    def din(name, shape, dtype):
        return nc.dram_tensor(name, shape, dtype, kind="ExternalInput").ap()

    p = {}
    # activations
    p["xt"] = din("xt", [D, N], BT)          # x[b].T, local tokens first
    p["xres"] = din("xres", [D, T], F32)     # f32 residual columns (local)
    p["cvec"] = din("cvec", [D, 1], F32)     # c[b]
    if FP8["ckv"]:
        p["srct8"] = din("srct8", [D, M], E4)
    else:
        p["srct"] = din("srct", [D, M], BT)
    # weights. fp8 ones are packed [KP*128, 2, F] (k-pairs), pre-scaled
    p["ada"] = din("ada", [D // 2, 2, NMOD * D], E4)

    def w_in(name, krows, fcols, fp8):
        if fp8:
            p[name] = din(name, [krows // 2, 2, fcols], E4)
        else:
            p[name] = din(name, [krows, fcols], BT)

    w_in("wqkv", D, 3 * D, FP8["qkv"])
    w_in("wo", D, D, FP8["wo"])
    w_in("wcq", D, D, FP8["cq"])
    w_in("wckv", D, 2 * D, FP8["ckv"])
    w_in("wco", D, D, FP8["co"])
    w_in("w1", D, MHP, FP8["ffn"])
    w_in("w2", D, MHP, FP8["ffn"])
    w_in("w3", MHP, D, FP8["ffn"])
    # feature-major f32 vectors [128, k]  (column j = feature tile j)
    p["adab"] = din("adab", [128, NMOD * DT], F32)
    p["n1w"] = din("n1w", [128, DT], F32)
    p["ncw"] = din("ncw", [128, DT], F32)
    p["n2w"] = din("n2w", [128, DT], F32)
    p["qkvb"] = din("qkvb", [128, 3 * DT], F32)
    p["obf"] = din("obf", [128, DT], F32)    # sa_o_b + v_bias @ Wo (host fold)
    p["cqb"] = din("cqb", [128, DT], F32)
    p["ckb"] = din("ckb", [128, DT], F32)    # cross-k bias
    p["cobf"] = din("cobf", [128, DT], F32)  # ca_o_b + cross-v bias @ Wco
    p["b1f"] = din("b1f", [128, FHT], F32)
    p["b2f"] = din("b2f", [128, FHT], F32)
    p["b3f"] = din("b3f", [128, DT], F32)
    # constant selector matrices
    p["ones128"] = din("ones128", [128, 128], BT)
    p["bd16"] = din("bd16", [128, 128], BT)
    p["eye32"] = din("eye32", [32, 32], F32)
    for nm in ("qsel", "ksel", "cqsel", "cksel"):
        p[nm] = din(nm, [16, D], BT)
    p["rsel"] = din("rsel", [16, D], F32)

    p["out"] = nc.dram_tensor("out", [D, T], F32, kind="ExternalOutput").ap()
    if DBG:
        for nm, shape in (("d_mods", [128, NMOD * DT]), ("d_den", [16, T]),
                          ("d_q0", [128, T]), ("d_k0", [128, N]),
                          ("d_v0", [128, 1040]), ("d_xn1", [128, 2048]),
                          ("d_pt0", [128, 1024]), ("d_o0", [128, 1024]),
                          ("d_x1", [128, T])):
            p[nm] = nc.dram_tensor(nm, shape, mybir.dt.float32
                                   if nm in ("d_mods", "d_den", "d_x1")
                                   else mybir.dt.bfloat16
                                   if nm in ("d_q0", "d_k0", "d_v0", "d_pt0")
                                   else mybir.dt.float8e4,
                                   kind="ExternalOutput").ap()

    with tile.TileContext(nc) as tc:
        _emit(nc, tc, p, mybir)
    nc.compile()
    return nc


def _emit(nc, tc, p, mybir):
    ALU = mybir.AluOpType
    ACTF = mybir.ActivationFunctionType
    PM = mybir.MatmulPerfMode
    F32 = mybir.dt.float32
    BT = mybir.dt.bfloat16
    E4 = mybir.dt.float8e4

    pg = tc.alloc_tile_pool(name="pg", bufs=1)
    ps = tc.alloc_tile_pool(name="ps", bufs=8, space="PSUM")

    # ---- psum allocators: 2x [128,1024] (2 banks) + 4x [128,512] ----
    def psum2(name):
        return ps.tile([128, 1024], F32, tag="ps2", name=name, bufs=2)

    def psum(name):
        return ps.tile([128, 512], F32, tag="ps1", name=name, bufs=4)

    full8 = all(FP8.values())

    # ---- sbuf allocators ----
    def bigw(name):   # wide bf16 activations (xt/k/v tiles)
        return pg.tile([128, 1040], BT, tag="bigw", name=name,
                       bufs=32 if full8 else 40)

    def xf(name):     # f32 [128, T] residual-stream tiles
        return pg.tile([128, T], F32, tag="xf", name=name, bufs=16)

    def qt(name):     # bf16 [128, T] q/o tiles
        return pg.tile([128, T], BT, tag="qt", name=name,
                       bufs=9 if full8 else 16)

    def wgt(name, wid=512, dt=None):  # weight stream tiles (ada chunks)
        return pg.tile([128, wid], dt or BT, tag="wgt", name=name,
                       bufs=6 if full8 else 9)

    def wgt8(name, wid):   # fp8 packed weight stream tiles
        return pg.tile([128, wid], E4, tag="wgt8", name=name, bufs=8)

    def pairw(name):       # fp8 k-pair tiles, 1024 tokens (srct / xn1)
        return pg.tile([128, 2048], E4, tag="pairw", name=name, bufs=8)

    def pair8(name, wid):  # fp8 k-pair tiles, 512 tokens (xn/o/h)
        return pg.tile([128, 1024], E4, tag="pair8", name=name, bufs=15)

    def ptile(name):  # wide exp(p) tiles / bf16 scratch
        return pg.tile([128, 1024], BT, tag="pt", name=name, bufs=4 if full8 else 3)

    def sqt(name):    # square scratch
        return pg.tile([128, 512], BT, tag="sq", name=name, bufs=3)

    def scratch4k(name, rows=128, wid=1024):  # f32 scratch (rr/ssq/den)
        return pg.tile([rows, wid], F32, tag="s4k", name=name, bufs=1)

    def scrbf(name, rows=16, wid=1024):
        return pg.tile([rows, wid], BT, tag="sbf", name=name, bufs=2)

    # =====================================================================
    # Stage 0: input DMAs in critical-path order, then silu(c)
    # =====================================================================
    xt_sb = []
    for k in range(DT):
        t = bigw(f"xt{k}")
        nc.sync.dma_start(t[:, 0:N], p["xt"][k * 128:(k + 1) * 128, :])
        xt_sb.append(t)
    cst = {}
    c_eps = pg.tile([128, 1], F32, tag="c_eps", name="c_eps")
    nc.any.memset(c_eps[:], EPS)
    for nm, k in (("ones128", 128), ("bd16", 128)):
        t = pg.tile([128, k], BT, tag=nm, name=f"c_{nm}")
        nc.sync.dma_start(t[:], p[nm][:])
        cst[nm] = t
    t = pg.tile([32, 32], F32, tag="eye32", name="c_eye32")
    nc.sync.dma_start(t[:], p["eye32"][:])
    cst["eye32"] = t
    for nm in ("qsel", "ksel", "cqsel", "cksel"):
        t = pg.tile([16, D], BT, tag=nm, name=f"c_{nm}")
        nc.sync.dma_start(t[:], p[nm][:])
        cst[nm] = t[:]
    t = pg.tile([16, D], F32, tag="rsel", name="c_rsel")
    nc.sync.dma_start(t[:], p["rsel"][:])
    cst["rsel"] = t[:]
    for nm, k in (("adab", NMOD * DT), ("n1w", DT), ("ncw", DT), ("n2w", DT),
                  ("qkvb", 3 * DT), ("obf", DT), ("cqb", DT), ("ckb", DT),
                  ("cobf", DT), ("b1f", FHT), ("b2f", FHT), ("b3f", DT)):
        t = pg.tile([128, k], F32, tag=nm, name=f"c_{nm}")
        nc.sync.dma_start(t[:], p[nm][:])
        cst[nm] = t

    dmy = pg.tile([1, 4], F32, tag="dmy", name="dmy")

    def prewarm(func):
        nc.scalar.activation(dmy[:, 0:1], c_eps[0:1, 0:1], func)

    cv = pg.tile([128, DT], F32, tag="cv", name="cv")
    nc.sync.dma_start(cv[:], p["cvec"][:].rearrange("(k p) o -> p (k o)", p=128))
    sc = pg.tile([128, DT], BT, tag="sc", name="sc")
    nc.scalar.activation(sc[:], cv[:], ACTF.Sigmoid)
    prewarm(ACTF.Sqrt)
    nc.vector.tensor_tensor(sc[:], sc[:], cv[:], ALU.mult)
    # silu(c) as fp8 DoubleRow stationary: per kp, the two k-halves at
    # byte offsets {0, 16} (weights AP needs 16B-aligned steps)
    sc8 = pg.tile([128, 128], E4, tag="sc8", name="sc8")
    sc8v = sc8[:].rearrange("p (kp two s) -> p kp two s", two=2, s=16)
    nc.vector.tensor_copy(sc8v[:, :, :, 0:1],
                          sc[:].rearrange("p (kp two) -> p kp two", two=2)
                          .rearrange("p kp two -> p kp two ()"))

    # =====================================================================
    # AdaLN mods: matvec silu(c) @ ada per group, strips gathered to
    # [18, 512] then PE-transposed to feature-major [128, 72].
    # Groups 0-2 (shift/scale/gate of SA) run up front; groups 3-8 are
    # emitted via a generator and drained as filler during stage 1.
    # =====================================================================
    mods = pg.tile([128, NMOD * DT], F32, tag="mods", name="mods")
    asmA = pg.tile([4, 512], F32, tag="asmA", name="asmA")    # groups 0-1
    asmB = pg.tile([8, 512], F32, tag="asmB", name="asmB")    # groups 2-5
    asmC = pg.tile([6, 512], F32, tag="asmC", name="asmC")    # groups 6-8

    def ada_group_units(groups, asm, gbase):
        for grp in groups:
            for ch in range(2):
                gi = grp * 2 + ch
                gtiles = []
                for kp in range(KP):
                    at = wgt(f"ada_g{gi}_{kp}", wid=1024, dt=E4)
                    nc.sync.dma_start(
                        at[:].rearrange("p (two f) -> p two f", two=2),
                        p["ada"][kp * 128:(kp + 1) * 128, :,
                                 grp * D + ch * 512:grp * D + (ch + 1) * 512])
                    gtiles.append(at)
                pm = psum(f"pm{gi}")
                for kp in range(KP):
                    nc.tensor.matmul(
                        pm[0:1, :], sc8v[:, kp, :, 0:1],
                        gtiles[kp][:].rearrange("p (two f) -> p two f", two=2),
                        start=(kp == 0), stop=(kp == KP - 1),
                        perf_mode=PM.DoubleRow)
                strip = pg.tile([1, 512], F32, tag="strip", name=f"str{gi}",
                                bufs=2)
                nc.scalar.activation(strip[:], pm[0:1, :], ACTF.Identity,
                                     scale=IWS)
                nc.sync.dma_start(asm[gi - 2 * gbase:gi - 2 * gbase + 1, :],
                                  strip[:])
                yield

    def ada_transpose(asm, g0, ng):
        """Transpose an assembly tile's rows into mods columns g0..g0+ng."""
        nr = 2 * ng
        dst = mods[:].rearrange("p (g c k) -> p g c k", c=2, k=4)
        for c4 in range(4):
            pt_ps = psum(f"modsT{g0}_{c4}")
            nc.tensor.transpose(pt_ps[0:128, 0:nr],
                                asm[0:nr, c4 * 128:(c4 + 1) * 128],
                                cst["eye32"][0:nr, 0:nr])
            src = pt_ps[0:128, 0:nr].rearrange("p (g c) -> p g c", c=2)
            nc.vector.tensor_tensor(
                dst[:, g0:g0 + ng, :, c4], src, cst["adab"][:].rearrange(
                    "p (g c k) -> p g c k", c=2, k=4)[:, g0:g0 + ng, :, c4],
                ALU.add)

    for _ in ada_group_units(range(2), asmA, 0):
        pass
    ada_transpose(asmA, 0, 2)
    ada_mid = ada_group_units(range(2, 6), asmB, 2)
    ada_late = ada_group_units(range(6, NMOD), asmC, 6)

    def msl(i):  # mods columns of modulation param i
        return mods[:, i * DT:(i + 1) * DT]

    def mk_seff(nm, i_scale, w):
        s1 = pg.tile([128, DT], F32, tag=f"seff_{nm}", name=f"seff_{nm}")
        nc.vector.tensor_scalar(s1[:], msl(i_scale), 1.0, None, ALU.add)
        nc.vector.tensor_tensor(s1[:], s1[:], cst[w][:], ALU.mult)
        return s1

    def mk_gb(nm, i_gate, bias):
        t = pg.tile([128, DT], F32, tag=f"gb_{nm}", name=f"gb_{nm}")
        nc.vector.tensor_tensor(t[:], msl(i_gate), cst[bias][:], ALU.mult)
        return t

    seff = {"sa": mk_seff("sa", 1, "n1w")}
    gb = {}
    sh_col = {"sa": 0, "ca": 3, "ff": 6}
    g_col = {"sa": 2, "ca": 5, "ff": 8}

    # =====================================================================
    # helpers
    # =====================================================================
    def load_wgroup(w_ap, cols0, cols, tagname):
        tiles = []
        for k in range(DT):
            t = wgt(f"{tagname}_{k}", wid=cols)
            nc.sync.dma_start(
                t[:, 0:cols], w_ap[k * 128:(k + 1) * 128, cols0:cols0 + cols])
            tiles.append(t)
        return tiles

    def norm_mod(xtiles, Ttok, seff_t, sh_slice, name, alloc, fp8_pairs):
        """RMS + AdaLN modulate of feature-major tiles.

        fp8_pairs=True -> returns KP pair tiles [128, 2*Ttok] fp8;
        else DT tiles via alloc()."""
        NCH = Ttok // 512
        pss = [psum(f"ssn_{name}{c}") for c in range(NCH)]
        for k in range(DT):
            for c in range(NCH):
                sq = sqt(f"sq_{name}{k}_{c}")
                nc.gpsimd.tensor_tensor(sq[:], xtiles[k][:, c * 512:(c + 1) * 512],
                                        xtiles[k][:, c * 512:(c + 1) * 512],
                                        ALU.mult)
                nc.tensor.matmul(pss[c][:], cst["ones128"][:], sq[:],
                                 start=(k == 0), stop=(k == DT - 1))
        rr = scratch4k(f"rr_{name}")
        for c in range(NCH):
            nc.scalar.activation(rr[:, c * 512:(c + 1) * 512], pss[c][:],
                                 ACTF.Sqrt, bias=c_eps[:], scale=1.0 / D)
        if fp8_pairs:
            palloc = pair8 if Ttok == 512 else (lambda nm, w: pairw(nm))
            xn = [palloc(f"xn_{name}{kp}", 2 * Ttok) for kp in range(KP)]
            dsts = [xn[k // 2][:, (k % 2) * Ttok:(k % 2 + 1) * Ttok]
                    for k in range(DT)]
        else:
            xn = [alloc(f"xn_{name}{k}") for k in range(DT)]
            dsts = [t[:, 0:Ttok] for t in xn]
        # c-chunked: downstream consumers of chunk 0 unblock ~3us earlier
        for c in range(NCH):
            cs = slice(c * 512, (c + 1) * 512)
            nc.vector.reciprocal_approx_fast(rr[:, cs], rr[:, cs])
            for k in range(DT):
                tmp = sqt(f"xm_{name}{k}_{c}")
                nc.vector.tensor_tensor(tmp[:], xtiles[k][:, cs],
                                        rr[:, cs], ALU.mult)
                nc.vector.tensor_scalar(
                    dsts[k].rearrange("p t -> p t")[:, cs] if False else
                    (xn[k // 2][:, (k % 2) * Ttok + c * 512:
                                (k % 2) * Ttok + (c + 1) * 512]
                     if fp8_pairs else xn[k][:, cs]),
                    tmp[:], seff_t[:, k:k + 1], sh_slice[:, k:k + 1],
                    ALU.mult, ALU.add)
        return xn

    def qk_norm_start(qtiles, Ttok, selname, name):
        """Per-head RMS norm stats; returns a generator of per-tile apply
        units (t-major) so callers can interleave them with other PE work.
        Head-norm weight is folded into the sel matrix."""
        NCH = Ttok // 512
        ssq = scratch4k(f"ssq_{name}", rows=16)
        for c in range(NCH):
            pq = psum(f"psq_{name}{c}")
            for t in range(DT):
                sq = sqt(f"qs_{name}{t}_{c}")
                nc.gpsimd.tensor_tensor(sq[:], qtiles[t][:, c * 512:(c + 1) * 512],
                                        qtiles[t][:, c * 512:(c + 1) * 512],
                                        ALU.mult)
                nc.tensor.matmul(pq[0:16, :],
                                 cst["bd16"][:, t * 16:(t + 1) * 16], sq[:],
                                 start=(t == 0), stop=(t == DT - 1))
            nc.scalar.activation(ssq[:, c * 512:(c + 1) * 512], pq[0:16, :],
                                 ACTF.Sqrt, bias=c_eps[0:16, :], scale=1.0 / HD)
        nc.vector.reciprocal_approx_fast(ssq[:, 0:Ttok], ssq[:, 0:Ttok])
        rqb = scrbf(f"rqb_{name}")
        nc.vector.tensor_copy(rqb[:, 0:Ttok], ssq[:, 0:Ttok])

        def apply_units():
            for t in range(DT):
                for c in range(NCH):
                    pb = psum(f"qb_{name}{t}_{c}")
                    nc.tensor.matmul(pb[:],
                                     cst[selname][:, t * 128:(t + 1) * 128],
                                     rqb[:, c * 512:(c + 1) * 512],
                                     start=True, stop=True)
                    nc.vector.tensor_tensor(qtiles[t][:, c * 512:(c + 1) * 512],
                                            qtiles[t][:, c * 512:(c + 1) * 512],
                                            pb[:], ALU.mult)
                yield

        return apply_units()

    def qk_norm(qtiles, Ttok, selname, name):
        for _ in qk_norm_start(qtiles, Ttok, selname, name):
            pass

    def attention(q_sb, k_sb, v_sb, Tk, name, filler=None, o_fp8=False):
        """softmax(q k^T / 8) v, pipelined per head-PAIR: the two heads of
        d-tile t are emitted as row-tiled concurrent K=64 score matmuls
        into one 2-bank psum tile, exponentiated with a single wide ACT
        op. P@V chases the exp stream; denominator = PSUM row 64 (ones
        column of v). Output fp8 k-pair tiles when o_fp8."""
        KTk = Tk // 128
        if o_fp8:
            o_pair = [pair8(f"o_{name}{tp}", 2 * T) for tp in range(KP)]

            def o_ap(t, lo=0, hi=128):
                return o_pair[t // 2][lo:hi, (t % 2) * T:(t % 2 + 1) * T]
        else:
            o_sb = [qt(f"o_{name}{t}") for t in range(DT)]

            def o_ap(t, lo=0, hi=128):
                return o_sb[t][lo:hi, :]
        rdb = pg.tile([16, T], F32, tag="rdbf", name=f"rdb_{name}")
        nc.vector.memset(rdb[:], 0.0)

        def normalize(t):
            pb = psum(f"ob_{name}{t}")
            nc.tensor.matmul(pb[:], cst["rsel"][:, t * 128:(t + 1) * 128],
                             rdb[:, 0:T], start=True, stop=True)
            nc.vector.tensor_tensor(o_ap(t), o_ap(t), pb[:], ALU.mult)

        for t in range(DT):
            pts = {}

            def s_unit(kt):
                s_ps = psum2(f"s_{name}{t}_{kt}")
                nc.tensor.matmul(
                    s_ps[:, 0:512], k_sb[t][0:64, kt * 128:(kt + 1) * 128],
                    q_sb[t][0:64, 0:T], start=True, stop=True)
                nc.tensor.matmul(
                    s_ps[:, 512:1024], k_sb[t][64:128, kt * 128:(kt + 1) * 128],
                    q_sb[t][64:128, 0:T], start=True, stop=True)
                pt = ptile(f"pt_{name}{t}_{kt}")
                nc.scalar.activation(pt[:], s_ps[:], ACTF.Exp, scale=ATT_SCALE)
                if DBG and name == "a1" and t == 0 and kt == 0:
                    nc.sync.dma_start(p["d_pt0"][:], pt[:])
                pts[kt] = pt

            # normalize pair t-1 first so its pb matmul grabs a ps1 slot
            # before this pair's poAB double-buffer claims two more
            if t >= 1:
                normalize(t - 1)
            poAB = [psum(f"po_{name}{2 * t}"), psum(f"po_{name}{2 * t + 1}")]
            s_unit(0)
            for kt in range(KTk):
                if kt + 1 < KTk:
                    s_unit(kt + 1)
                pt = pts.pop(kt)
                for half in range(2):
                    h16 = 2 * t + half
                    nc.tensor.matmul(poAB[half][0:65, :],
                                     v_sb[kt][:, h16 * 65:(h16 + 1) * 65],
                                     pt[:, half * 512:(half + 1) * 512],
                                     start=(kt == 0), stop=(kt == KTk - 1))
            for half in range(2):
                h16 = 2 * t + half
                lo = 64 * half
                if o_fp8:
                    # store o/64 in fp8 (unnormalized P@V can exceed fp8
                    # range); the x64 is folded into the rsel matrix
                    nc.vector.tensor_scalar(o_ap(t, lo, lo + 64),
                                            poAB[half][0:64, :],
                                            1.0 / 64.0, None, ALU.mult)
                else:
                    nc.vector.tensor_copy(o_ap(t, lo, lo + 64),
                                          poAB[half][0:64, :])
                dstrip = pg.tile([1, 512], F32, tag="strip",
                                 name=f"dstr_{name}{h16}", bufs=2)
                nc.vector.tensor_copy(dstrip[:, 0:T], poAB[half][64:65, :])
                nc.vector.reciprocal_approx_fast(dstrip[:, 0:T],
                                                 dstrip[:, 0:T])
                nc.sync.dma_start(rdb[h16:h16 + 1, :], dstrip[:, 0:T])
            if filler is not None:
                filler(t)
        normalize(DT - 1)

        if DBG and name == "a1":
            nc.sync.dma_start(p["d_den"][:], rdb[:])
        if DBG and name == "a1" and o_fp8:
            nc.sync.dma_start(p["d_o0"][:], o_pair[0][:])
        return o_pair if o_fp8 else o_sb

    # ---- projection helpers ----
    def load_w8group(w_ap, cols0, cols, tagname):
        """fp8 pair tiles: KP x [128, 2*cols]."""
        tiles = []
        for kp in range(KP):
            t = wgt8(f"{tagname}_{kp}", 2 * cols)
            nc.sync.dma_start(
                t[:].rearrange("p (two f) -> p two f", two=2),
                w_ap[kp * 128:(kp + 1) * 128, :, cols0:cols0 + cols])
            tiles.append(t)
        return tiles

    def mm_dr(pp, wt, f, xp, Tt, c, kp, nkp=KP):
        """One DoubleRow accumulation step."""
        lhsT = wt[kp][:].rearrange("p (two f) -> p two f", two=2)[
            :, :, f * 128:(f + 1) * 128]
        rhs = xp[kp][:].rearrange("p (two t) -> p two t", two=2)[
            :, :, c * 512:(c + 1) * 512]
        nc.tensor.matmul(pp[:], lhsT, rhs, start=(kp == 0), stop=(kp == nkp - 1),
                         perf_mode=PM.DoubleRow)

    def proj_fm8(w_ap, wcols0, xp, Tt, bias, bias0, name, alloc, n_f=DT):
        """fp8-DoubleRow feature-major projection over n_f output tiles."""
        outs = []
        NCH = Tt // 512
        for f0 in range(0, n_f, 8):
            nf = min(8, n_f - f0)
            wt = load_w8group(w_ap, wcols0 + f0 * 128, nf * 128, f"{name}_w{f0}")
            for f in range(nf):
                o = alloc(f"{name}_o{f0 + f}")
                outs.append(o)
                for c in range(NCH):
                    pp = psum(f"p_{name}{f0 + f}_{c}")
                    for kp in range(KP):
                        mm_dr(pp, wt, f, xp, Tt, c, kp)
                    nc.scalar.activation(
                        o[:, c * 512:(c + 1) * 512], pp[:], ACTF.Identity,
                        bias=bias[:, bias0 + f0 + f:bias0 + f0 + f + 1],
                        scale=IWS)
        return outs

    def proj_fm(wap, wcols0, xn, Tt, bias, bias0, name, alloc, n_f=DT):
        """bf16 feature-major projection (fallback path)."""
        outs = []
        NCH = Tt // 512
        for f0 in range(0, n_f, 8):
            nf = min(8, n_f - f0)
            wt = load_wgroup(wap, wcols0 + f0 * 128, nf * 128, f"{name}_w{f0}")
            for f in range(nf):
                o = alloc(f"{name}_o{f0 + f}")
                outs.append(o)
                for c in range(NCH):
                    pp = psum(f"p_{name}{f0 + f}_{c}")
                    for k in range(DT):
                        nc.tensor.matmul(pp[:], wt[k][:, f * 128:(f + 1) * 128],
                                         xn[k][:, c * 512:(c + 1) * 512],
                                         start=(k == 0), stop=(k == DT - 1))
                    nc.scalar.activation(
                        o[:, c * 512:(c + 1) * 512], pp[:], ACTF.Identity,
                        bias=bias[:, bias0 + f0 + f:bias0 + f0 + f + 1])
        return outs

    def vdst(o, c):
        return o[:, c * 8 * 65:(c * 8 + 8) * 65].rearrange(
            "p (g e) -> p g e", g=8)[:, :, 0:64]

    def vones(o):
        return o[:, 0:16 * 65].rearrange("p (g e) -> p g e", g=16)[:, :, 64:65]

    def proj_tok8(w_ap, wcols0, xp, name, interleave=None):
        """fp8-DoubleRow token-major V projection (ones cols interleaved)."""
        outs = []
        wt = load_w8group(w_ap, wcols0, D, f"{name}_w")
        for tt in range(N // 128):
            o = bigw(f"{name}_v{tt}")
            nc.any.memset(vones(o), 1.0)
            outs.append(o)
            for c in range(2):
                pp = psum(f"pv_{name}{tt}_{c}")
                for kp in range(KP):
                    lhsT = xp[kp][:].rearrange("p (two t) -> p two t", two=2)[
                        :, :, tt * 128:(tt + 1) * 128]
                    rhs = wt[kp][:].rearrange("p (two f) -> p two f", two=2)[
                        :, :, c * 512:(c + 1) * 512]
                    nc.tensor.matmul(pp[:], lhsT, rhs, start=(kp == 0),
                                     stop=(kp == KP - 1), perf_mode=PM.DoubleRow)
                nc.scalar.activation(vdst(o, c), pp[:].rearrange(
                    "p (g e) -> p g e", g=8), ACTF.Identity, scale=IWS)
                if interleave is not None:
                    interleave()
        return outs

    def proj_tok(wap, wcols0, xn, name):
        """bf16 token-major V projection."""
        wt = load_wgroup(wap, wcols0, D, f"{name}_w")
        outs = []
        for tt in range(N // 128):
            o = bigw(f"{name}_v{tt}")
            nc.any.memset(vones(o), 1.0)
            outs.append(o)
            for c in range(2):
                pp = psum(f"pv_{name}{tt}_{c}")
                for k in range(DT):
                    nc.tensor.matmul(pp[:], xn[k][:, tt * 128:(tt + 1) * 128],
                                     wt[k][:, c * 512:(c + 1) * 512],
                                     start=(k == 0), stop=(k == DT - 1))
                nc.scalar.activation(vdst(o, c), pp[:].rearrange(
                    "p (g e) -> p g e", g=8), ACTF.Identity)
        return outs

    def out_proj(w_key, fp8_on, o_in, resid, gcol, gbt, name, outalloc):
        """o @ Wo + gated residual add -> f32 tiles."""
        if fp8_on:
            wt = load_w8group(p[w_key], 0, D, f"{name}_w")
        else:
            wt = load_wgroup(p[w_key], 0, D, f"{name}_w")
        outs = []
        for f in range(DT):
            pp = psum(f"p{name}_{f}")
            if fp8_on:
                for kp in range(KP):
                    mm_dr(pp, wt, f, o_in, T, 0, kp)
            else:
                for k in range(DT):
                    nc.tensor.matmul(pp[:], wt[k][:, f * 128:(f + 1) * 128],
                                     o_in[k][:], start=(k == 0), stop=(k == DT - 1))
            xo = outalloc(f"{name}x{f}")
            sc_col = gcol[:, f:f + 1]
            if fp8_on:
                scol = pg.tile([128, 1], F32, tag="gsc", name=f"gs{name}{f}",
                               bufs=4)
                nc.vector.tensor_scalar(scol[:], sc_col, IWS, None, ALU.mult)
                sc_col = scol[:]
            nc.vector.affine_then_add(xo[:], pp[:], resid[f][:],
                                      sc_col, gbt[:, f:f + 1])
            outs.append(xo)
        return outs

    # =====================================================================
    # Stage 0b: source tokens load; cross-attention K/V projections are
    # emitted as filler inside attention-1.
    # =====================================================================
    srct_p = []

    def load_srct():
        if not FP8["ckv"]:
            return
        for kp in range(KP):
            t = pairw(f"srct{kp}")
            for j in range(2):
                nc.sync.dma_start(
                    t[:, j * M:(j + 1) * M],
                    p["srct8"][(2 * kp + j) * 128:(2 * kp + j + 1) * 128, :])
            srct_p.append(t)
    if not FP8["ckv"]:
        srct_sb = []
        for k in range(DT):
            t = bigw(f"srct{k}")
            nc.sync.dma_start(t[:, 0:M], p["srct"][k * 128:(k + 1) * 128, :])
            srct_sb.append(t)

    kca = [bigw(f"kca_o{f}") for f in range(DT)]
    vca = []
    for tt in range(M // 128):
        o = bigw(f"vca_v{tt}")
        nc.any.memset(vones(o), 1.0)
        vca.append(o)

    def ckv_units():
        """Generator of one-psum-group units of cross-KV projection work."""
        if FP8["ckv"]:
            wt = load_w8group(p["wckv"], 0, D, "kca_w")
            for f in range(DT):
                for c in range(2):
                    pp = psum(f"p_kca{f}_{c}")
                    for kp in range(KP):
                        mm_dr(pp, wt, f, srct_p, M, c, kp)
                    nc.vector.tensor_scalar(kca[f][:, c * 512:(c + 1) * 512],
                                            pp[:], IWS, cst["ckb"][:, f:f + 1],
                                            ALU.mult, ALU.add)
                    yield
            wtv = load_w8group(p["wckv"], D, D, "vca_w")
            for tt in range(M // 128):
                for c in range(2):
                    pp = psum(f"pv_vca{tt}_{c}")
                    for kp in range(KP):
                        lhsT = srct_p[kp][:].rearrange(
                            "p (two t) -> p two t", two=2)[:, :, tt * 128:(tt + 1) * 128]
                        rhs = wtv[kp][:].rearrange(
                            "p (two f) -> p two f", two=2)[:, :, c * 512:(c + 1) * 512]
                        nc.tensor.matmul(pp[:], lhsT, rhs, start=(kp == 0),
                                         stop=(kp == KP - 1), perf_mode=PM.DoubleRow)
                    nc.vector.tensor_scalar(vdst(vca[tt], c), pp[:].rearrange(
                        "p (g e) -> p g e", g=8), IWS, None, ALU.mult)
                    yield
        else:
            wt = load_wgroup(p["wckv"], 0, D, "kca_w")
            for f in range(DT):
                for c in range(2):
                    pp = psum(f"p_kca{f}_{c}")
                    for k in range(DT):
                        nc.tensor.matmul(pp[:], wt[k][:, f * 128:(f + 1) * 128],
                                         srct_sb[k][:, c * 512:(c + 1) * 512],
                                         start=(k == 0), stop=(k == DT - 1))
                    nc.scalar.activation(kca[f][:, c * 512:(c + 1) * 512], pp[:],
                                         ACTF.Identity, bias=cst["ckb"][:, f:f + 1])
                    yield
            wtv = load_wgroup(p["wckv"], D, D, "vca_w")
            for tt in range(M // 128):
                for c in range(2):
                    pp = psum(f"pv_vca{tt}_{c}")
                    for k in range(DT):
                        nc.tensor.matmul(pp[:], srct_sb[k][:, tt * 128:(tt + 1) * 128],
                                         wtv[k][:, c * 512:(c + 1) * 512],
                                         start=(k == 0), stop=(k == DT - 1))
                    nc.scalar.activation(vdst(vca[tt], c), pp[:].rearrange(
                        "p (g e) -> p g e", g=8), ACTF.Identity)
                    yield

    ckv_gen = None

    ckv_box = []

    def a1_filler(t):
        for _ in range(5):
            next(ckv_box[0], None)
        next(ada_mid, None)

    def a2_filler(t):
        next(ada_late, None)

    # =====================================================================
    # Stage 1: self-attention sublayer
    # =====================================================================
    xn1 = norm_mod(xt_sb, N, seff["sa"], msl(sh_col["sa"]), "n1",
                   bigw, FP8["qkv"])
    if FP8["qkv"]:
        q_sa = proj_fm8(p["wqkv"], 0, xn1, T, cst["qkvb"], 0, "qsa", qt)
        k_sa = proj_fm8(p["wqkv"], D, xn1, N, cst["qkvb"], DT, "ksa", bigw)
        qknq = qk_norm_start(q_sa, T, "qsel", "qsa")
        qknk = qk_norm_start(k_sa, N, "ksel", "ksa")

        def qk_drain():
            next(qknq, None)
            next(qknk, None)
        v_sa = proj_tok8(p["wqkv"], 2 * D, xn1, "vsa", interleave=qk_drain)
        for g in (qknq, qknk):
            for _ in g:
                pass
    else:
        q_sa = proj_fm(p["wqkv"], 0, xn1, T, cst["qkvb"], 0, "qsa", qt)
        k_sa = proj_fm(p["wqkv"], D, xn1, N, cst["qkvb"], DT, "ksa", bigw)
        v_sa = proj_tok(p["wqkv"], 2 * D, xn1, "vsa")
        qk_norm(q_sa, T, "qsel", "qsa")
        qk_norm(k_sa, N, "ksel", "ksa")
    if DBG:
        nc.sync.dma_start(p["d_xn1"][:], xn1[0][:])
        nc.sync.dma_start(p["d_v0"][:], v_sa[0][:])
    if DBG:
        nc.sync.dma_start(p["d_q0"][:], q_sa[0][:])
        nc.sync.dma_start(p["d_k0"][:], k_sa[0][:, 0:N])
    load_srct()
    ckv_box.append(ckv_units())
    xres_sb = []
    for k in range(DT):
        t = xf(f"xres{k}")
        nc.sync.dma_start(t[:], p["xres"][k * 128:(k + 1) * 128, :])
        xres_sb.append(t)
    prewarm(ACTF.Exp)
    o1 = attention(q_sa, k_sa, v_sa, N, "a1", filler=a1_filler,
                   o_fp8=FP8["wo"])
    for _ in ckv_box[0]:
        pass
    for _ in ada_mid:
        pass
    prewarm(ACTF.Sqrt)
    ada_transpose(asmB, 2, 4)
    gb["sa"] = mk_gb("sa", 2, "obf")
    seff["ca"] = mk_seff("ca", 4, "ncw")
    gb["ca"] = mk_gb("ca", 5, "cobf")

    x1 = out_proj("wo", FP8["wo"], o1, xres_sb, msl(g_col["sa"]), gb["sa"][:],
                  "o1", xf)
    if DBG:
        nc.sync.dma_start(p["d_x1"][:], x1[0][:])

    # =====================================================================
    # Stage 2: cross-attention sublayer
    # =====================================================================
    qknkca = qk_norm_start(kca, M, "cksel", "kca")
    xnc = norm_mod(x1, T, seff["ca"], msl(sh_col["ca"]), "nc", qt, FP8["cq"])
    if FP8["cq"]:
        q_ca = proj_fm8(p["wcq"], 0, xnc, T, cst["cqb"], 0, "qca", qt)
    else:
        q_ca = proj_fm(p["wcq"], 0, xnc, T, cst["cqb"], 0, "qca", qt)
    qknqca = qk_norm_start(q_ca, T, "cqsel", "qca")
    # tile-0 norms must precede a2 pair 0; later tiles drain in the filler
    next(qknkca, None)
    next(qknqca, None)

    def a2_filler2(t):
        next(qknkca, None)
        next(qknqca, None)
        a2_filler(t)
    prewarm(ACTF.Exp)
    o2 = attention(q_ca, kca, vca, M, "a2", filler=a2_filler2,
                   o_fp8=FP8["co"])
    for g in (qknkca, qknqca):
        for _ in g:
            pass
    for _ in ada_late:
        pass
    prewarm(ACTF.Sqrt)
    ada_transpose(asmC, 6, 3)
    if DBG:
        nc.sync.dma_start(p["d_mods"][:], mods[:])
    seff["ff"] = mk_seff("ff", 7, "n2w")
    gb["ff"] = mk_gb("ff", 8, "b3f")
    x2 = out_proj("wco", FP8["co"], o2, x1, msl(g_col["ca"]), gb["ca"][:],
                  "o2", xf)

    # =====================================================================
    # Stage 3: SwiGLU FFN sublayer
    # =====================================================================
    xn2 = norm_mod(x2, T, seff["ff"], msl(sh_col["ff"]), "n2", qt, FP8["ffn"])
    prewarm(ACTF.Silu)
    if FP8["ffn"]:
        NKH = FHT // 2  # 11 h k-pairs
        h_p8 = [pair8(f"h8_{fp}", 2 * T) for fp in range(NKH)]
        gsc = pg.tile([128, DT], F32, tag="gsc_ff", name="gsc_ff")
        nc.vector.tensor_scalar(gsc[:], msl(g_col["ff"]), IWS, None, ALU.mult)

        def w3_mm(fg, psf, kp):
            w3t = wgt8(f"w3_{fg}_{kp}", 2 * 512)
            nc.sync.dma_start(
                w3t[:].rearrange("p (two f) -> p two f", two=2),
                p["w3"][kp * 128:(kp + 1) * 128, :, fg * 128:(fg + 4) * 128])
            for ff in range(4):
                lhsT = w3t[:].rearrange("p (two f) -> p two f", two=2)[
                    :, :, ff * 128:(ff + 1) * 128]
                rhs = h_p8[kp][:].rearrange("p (two t) -> p two t", two=2)
                nc.tensor.matmul(psf[ff // 2][:, (ff % 2) * 512:(ff % 2 + 1) * 512],
                                 lhsT, rhs, start=(kp == 0),
                                 stop=(kp == NKH - 1),
                                 perf_mode=PM.DoubleRow)

        def w3_evac(fg, psf):
            for ff in range(4):
                f = fg + ff
                xo = xf(f"xout{f}")
                nc.vector.affine_then_add(
                    xo[:], psf[ff // 2][:, (ff % 2) * 512:(ff % 2 + 1) * 512],
                    x2[f][:], gsc[:, f:f + 1], gb["ff"][:, f:f + 1])
                nc.sync.dma_start(p["out"][f * 128:(f + 1) * 128, :], xo[:])

        # first w3 output half accumulates as h pairs complete
        psfA = [psum2(f"pfA{i}") for i in range(2)]
        for f0 in range(0, FHT, 8):
            nf = min(8, FHT - f0)
            w1t = load_w8group(p["w1"], f0 * 128, nf * 128, f"w1_{f0}")
            w2t = load_w8group(p["w2"], f0 * 128, nf * 128, f"w2_{f0}")
            for f in range(nf):
                fa = f0 + f
                pp1 = psum(f"ph1_{fa}")
                for kp in range(KP):
                    mm_dr(pp1, w1t, f, xn2, T, 0, kp)
                h1 = pg.tile([128, T], BT, tag="h_sb", name=f"h_{fa}", bufs=3)
                nc.scalar.activation(h1[:], pp1[:], ACTF.Silu,
                                     bias=cst["b1f"][:, fa:fa + 1], scale=IWS)
                pp2 = psum(f"ph2_{fa}")
                for kp in range(KP):
                    mm_dr(pp2, w2t, f, xn2, T, 0, kp)
                h2 = sqt(f"h2_{fa}")
                nc.vector.tensor_scalar(h2[:], pp2[:], IWS,
                                        cst["b2f"][:, fa:fa + 1],
                                        ALU.mult, ALU.add)
                nc.vector.tensor_tensor(
                    h_p8[fa // 2][:, (fa % 2) * T:(fa % 2 + 1) * T],
                    h1[:], h2[:], ALU.mult)
                if fa % 2 == 1:
                    w3_mm(0, psfA, fa // 2)
        w3_evac(0, psfA)
        psfB = [psum2(f"pfB{i}") for i in range(2)]
        for kp in range(NKH):
            w3_mm(4, psfB, kp)
        w3_evac(4, psfB)
    else:
        h_sb = []
        for f0 in range(0, FHT, 4):
            nf = min(4, FHT - f0)
            w1t = load_wgroup(p["w1"], f0 * 128, nf * 128, f"w1_{f0}")
            w2t = load_wgroup(p["w2"], f0 * 128, nf * 128, f"w2_{f0}")
            for f in range(nf):
                fa = f0 + f
                pp1 = psum(f"ph1_{fa}")
                for k in range(DT):
                    nc.tensor.matmul(pp1[:], w1t[k][:, f * 128:(f + 1) * 128],
                                     xn2[k][:], start=(k == 0), stop=(k == DT - 1))
                h1 = pg.tile([128, T], BT, tag="h_sb", name=f"h_{fa}", bufs=FHT)
                nc.scalar.activation(h1[:], pp1[:], ACTF.Silu,
                                     bias=cst["b1f"][:, fa:fa + 1])
                h_sb.append(h1)
                pp2 = psum(f"ph2_{fa}")
                for k in range(DT):
                    nc.tensor.matmul(pp2[:], w2t[k][:, f * 128:(f + 1) * 128],
                                     xn2[k][:], start=(k == 0), stop=(k == DT - 1))
                h2 = sqt(f"h2_{fa}")
                nc.scalar.activation(h2[:], pp2[:], ACTF.Identity,
                                     bias=cst["b2f"][:, fa:fa + 1])
                nc.vector.tensor_tensor(h1[:], h1[:], h2[:], ALU.mult)

        for fg in range(0, DT, 4):
            psf = [psum2(f"pf{fg + ff}") for ff in range(2)]
            for k in range(FHT):
                w3t = wgt(f"w3_{fg}_{k}", wid=512)
                nc.sync.dma_start(w3t[:],
                                  p["w3"][k * 128:(k + 1) * 128,
                                          fg * 128:(fg + 4) * 128])
                for ff in range(4):
                    nc.tensor.matmul(
                        psf[ff // 2][:, (ff % 2) * 512:(ff % 2 + 1) * 512],
                        w3t[:, ff * 128:(ff + 1) * 128],
                        h_sb[k][:], start=(k == 0), stop=(k == FHT - 1))
            for ff in range(4):
                f = fg + ff
                xo = xf(f"xout{f}")
                nc.vector.affine_then_add(
                    xo[:], psf[ff // 2][:, (ff % 2) * 512:(ff % 2 + 1) * 512],
                    x2[f][:], msl(g_col["ff"])[:, f:f + 1], gb["ff"][:, f:f + 1])
                nc.sync.dma_start(p["out"][f * 128:(f + 1) * 128, :], xo[:])

    pg.release()
    ps.release()


# ==========================================================================
# host side
# ==========================================================================

def _fm(vec):
    """[128*k] f32 vector -> feature-major [128, k] (col j = feature tile j)."""
    v = np.asarray(vec, np.float32)
    return np.ascontiguousarray(v.reshape(-1, 128).T)


def _bd16():
    bd = np.zeros((128, 128), np.float32)
    for t in range(8):
        for p_ in range(128):
            bd[p_, t * 16 + 2 * t + p_ // 64] = 1.0
    return bd.astype(BF16)


def _sel(weights64):
    """[16, 1024] selector: sel[i, t*128+p] = w[p%64] * (i == 2t + p//64)."""
    w = np.ones(64, np.float32) if weights64 is None else \
        np.asarray(weights64, np.float32)
    s = np.zeros((16, D), np.float32)
    for col in range(D):
        i = 2 * (col // 128) + (col % 128) // 64
        s[i, col] = w[col % 64]
    return s.astype(BF16)


def _stack32(mats, rows):
    out = np.zeros((rows, D), np.float32).astype(BF16)
    for i, m in enumerate(mats):
        out[32 * i:32 * i + 16] = m
    return out


def _pack8(w):
    """[K, F] f32 -> [K//2, 2, F] fp8 k-pair pack, scaled by WS."""
    w = np.asarray(w, np.float32) * WS
    w = np.clip(w, -239.0, 239.0)
    nkp = w.shape[0] // 256
    kp = w.reshape(nkp, 2, 128, -1).transpose(0, 2, 1, 3)  # [nkp,128,2,F]
    return np.ascontiguousarray(kp.reshape(nkp * 128, 2, w.shape[1])).astype(F8)


def make_in_maps(inputs):
    f32 = lambda a: np.ascontiguousarray(np.asarray(a, np.float32))
    bf = lambda a: np.ascontiguousarray(np.asarray(a, np.float32)).astype(BF16)

    x = f32(inputs["x"]); src = f32(inputs["source_tokens"]); c = f32(inputs["c"])
    qkv_b = f32(inputs["sa_qkv_b"])
    o_w = f32(inputs["sa_o_w"]); o_b = f32(inputs["sa_o_b"])
    ckv_b = f32(inputs["ca_kv_b"])
    co_w = f32(inputs["ca_o_w"]); co_b = f32(inputs["ca_o_b"])
    w1 = f32(inputs["mlp_w1"]); b1 = f32(inputs["mlp_b1"])
    w2 = f32(inputs["mlp_w2"]); b2 = f32(inputs["mlp_b2"])
    w3 = f32(inputs["mlp_w3"]); b3 = f32(inputs["mlp_b3"])

    # pad SwiGLU hidden to 2816; zero pads keep silu(0)*0 == 0 exact
    w1p = np.zeros((D, MHP), np.float32); w1p[:, :MH] = w1
    w2p = np.zeros((D, MHP), np.float32); w2p[:, :MH] = w2
    w3p = np.zeros((MHP, D), np.float32); w3p[:MH, :] = w3
    b1p = np.zeros(MHP, np.float32); b1p[:MH] = b1
    b2p = np.zeros(MHP, np.float32); b2p[:MH] = b2

    # fold the V biases through the linear attention + output projection:
    # softmax(..) @ (v + vb) @ Wo = softmax(..) @ v @ Wo + vb @ Wo
    obf = qkv_b[2 * D:3 * D] @ o_w + o_b
    cobf = ckv_b[D:2 * D] @ co_w + co_b

    shared = dict(
        ada=_pack8(f32(inputs["ada_w"])),
        adab=_fm(f32(inputs["ada_b"])), n1w=_fm(f32(inputs["n1_w"])),
        ncw=_fm(f32(inputs["nc_w"])), n2w=_fm(f32(inputs["n2_w"])),
        qkvb=_fm(qkv_b), obf=_fm(obf), cqb=_fm(f32(inputs["ca_q_b"])),
        ckb=_fm(ckv_b[0:D]), cobf=_fm(cobf),
        b1f=_fm(b1p), b2f=_fm(b2p), b3f=_fm(b3),
        ones128=np.ones((128, 128), BF16),
        bd16=_bd16(),
        eye32=np.eye(32, dtype=np.float32),
        qsel=_sel(inputs["sa_qn_w"]), ksel=_sel(inputs["sa_kn_w"]),
        cqsel=_sel(inputs["ca_qn_w"]), cksel=_sel(inputs["ca_kn_w"]),
        rsel=np.asarray(_sel(np.full(64, 64.0, np.float32)
                             if (FP8["wo"] and FP8["co"]) else None),
                        np.float32),
    )
    wsrc = dict(wqkv=f32(inputs["sa_qkv_w"]), wo=o_w,
                wcq=f32(inputs["ca_q_w"]), wckv=f32(inputs["ca_kv_w"]),
                wco=co_w, w1=w1p, w2=w2p, w3=w3p)
    for nm, key in (("wqkv", "qkv"), ("wo", "wo"), ("wcq", "cq"),
                    ("wckv", "ckv"), ("wco", "co")):
        shared[nm] = _pack8(wsrc[nm]) if FP8[key] else bf(wsrc[nm])
    for nm in ("w1", "w2", "w3"):
        shared[nm] = _pack8(wsrc[nm]) if FP8["ffn"] else bf(wsrc[nm])

    in_maps = []
    for cidx in range(NCORES):
        b, half = divmod(cidx, 2)
        xT = x[b].T  # [D, N]
        if half:
            xTp = np.concatenate([xT[:, T:], xT[:, :T]], axis=1)
        else:
            xTp = xT
        m = dict(shared)
        m["xt"] = np.ascontiguousarray(xTp).astype(BF16)
        m["xres"] = np.ascontiguousarray(xTp[:, :T])
        if FP8["ckv"]:
            m["srct8"] = np.clip(np.ascontiguousarray(src[b].T),
                                 -239.0, 239.0).astype(F8)
        else:
            m["srct"] = np.ascontiguousarray(src[b].T).astype(BF16)
        m["cvec"] = np.ascontiguousarray(c[b].reshape(D, 1))
        in_maps.append(m)
    return in_maps


def assemble(results):
    out = np.empty((B, N, D), np.float32)
    for cidx in range(NCORES):
        b, half = divmod(cidx, 2)
        out[b, half * T:(half + 1) * T, :] = results[cidx]["out"].T
    return out


_NC_CACHE = []


def kernel(**inputs):
    from concourse.bass_utils import run_bass_kernel_spmd
    if not _NC_CACHE:
        _NC_CACHE.append(build_graph())
    nc = _NC_CACHE[0]
    in_maps = make_in_maps(inputs)
    res = run_bass_kernel_spmd(nc, in_maps, core_ids=list(range(NCORES)))
    return assemble(res.results)


if __name__ == "__main__":
    nc = build_graph()
    print("graph built OK; instructions:",
          sum(len(bb.instructions) for bb in nc.main_func.blocks))
